# revision 1
# baseline (speedup 1.0000x reference)
"""Trainium2 Bass kernel for sparse CausalSelfAttention (8 full heads W=1024,
8 reduced-qk heads W=256), SPMD over 8 NeuronCores.

Sharding: core c -> batch c//4, head-group g=c%4 (full heads 2g,2g+1 and
reduced heads 2g,2g+1). Each core computes its QKV projection slices, windowed
attention in transposed layout, and a partial c_proj against its 256-row slice
of w_proj. Host sums the 4 partials per batch element.
"""

import numpy as np

import concourse.bacc as bacc
import concourse.mybir as mybir
from concourse import bass_utils
from concourse.tile import TileContext

# problem constants (hardcoded; kernel.py must be self-contained)
B, T, C = 2, 2048, 1024
HDIM = 64          # full head dim (and v dim of reduced heads)
RDIM = 32          # reduced qk dim
WF, WR = 1024, 256  # windows
QF, QR = 512, 256   # query-block sizes
N_CORES = 8
NK = C // 128       # k-tiles over C contraction

F32R = mybir.dt.float32r
F32 = mybir.dt.float32
BF16 = mybir.dt.bfloat16

# full-head mask offsets d = i0 - j0 (Q=512, W=1024): 1.0 where 0 <= d+f-p < W
MASKF_D = [0, -128, -256, -384, 640, 768, 896, 1024]
# reduced-head mask offsets (Q=256, W=256)
MASKR_D = [256, 128, 0, -128]
CLEAN_F = {128, 256, 384, 512}  # fully-valid offsets for full heads


def _full_mask_idx(d):
    if d in CLEAN_F:
        return None
    if d <= 0:
        return -d // 128
    return 4 + (d - 640) // 128


def _make_mask(nc, dst, d, w):
    """dst[p, f] = 1.0 where 0 <= d + f - p < w else 0.0 (on gpsimd)."""
    q = dst.shape[-1]
    nc.gpsimd.memset(dst, 1.0)
    # keep where (d + f - p) >= 0
    nc.gpsimd.affine_select(out=dst, in_=dst, compare_op=mybir.AluOpType.is_ge,
                            fill=0.0, base=d, pattern=[[1, q]],
                            channel_multiplier=-1)
    # keep where (w - 1 - d + p - f) >= 0  i.e. d + f - p < w
    nc.gpsimd.affine_select(out=dst, in_=dst, compare_op=mybir.AluOpType.is_ge,
                            fill=0.0, base=w - 1 - d, pattern=[[-1, q]],
                            channel_multiplier=1)


def _emit_body(nc, pools, aps):
    wpool, xpool, xbpool, qkpool, ppool, opool, rpool, ps_misc, ps_s, ps_y = pools
    xT, wq, wk, wqr, wkr, wv, wproj, out = aps

    # ---- weight tiles; DMAs issued after xt block 0 (fast PE start) ----
    wq_sb = wpool.tile([128, NK, 128], F32R, tag="wq")
    wk_sb = wpool.tile([128, NK, 128], F32R, tag="wk")
    wqr_sb = wpool.tile([128, NK, 128], F32R, tag="wqr")
    wkr_sb = wpool.tile([128, NK, 128], F32R, tag="wkr")
    wv_sb = wpool.tile([128, NK, 256], F32R, tag="wv")
    wproj_sb = wpool.tile([128, 2, C], F32R, tag="wproj")

    # ---- masks generated on gpsimd (keeps the DMA queue for x/weights) ----
    mf_sb = wpool.tile([128, len(MASKF_D), QF], BF16, tag="mf")
    mr_sb = wpool.tile([128, len(MASKR_D), QR], BF16, tag="mr")
    for m, d in enumerate(MASKF_D):
        _make_mask(nc, mf_sb[:, m, :], d, WF)
    for m, d in enumerate(MASKR_D):
        _make_mask(nc, mr_sb[:, m, :], d, WR)

    # persistent transposed activations [dim-stack, T]
    qTf = qkpool.tile([128, T], F32R, tag="qTf")  # rows: hA q (64) | hB q (64)
    kTf = qkpool.tile([128, T], F32R, tag="kTf")
    qTr = qkpool.tile([128, T], F32R, tag="qTr")  # rows: rA qr|0|rB qr|0
    kTr = qkpool.tile([128, T], F32R, tag="kTr")
    # v values + ones block: [128, T-tile, head, 128] (cols 64:128 = 1.0)
    v_sb = qkpool.tile([128, T // 128, 4, 128], BF16, tag="v")
    nc.gpsimd.memset(v_sb[:, :, :, 64:128], 1.0)
    # attention outputs yT (normalized), stacked per pair
    yTf = qkpool.tile([128, T], F32R, tag="yTf")
    yTr = qkpool.tile([128, T], F32R, tag="yTr")

    # ---- phase B: projections, streaming xT by T-block of 512 ----
    xT3 = xT.rearrange("(k p) t -> p k t", p=128)
    for tb in range(T // 512):
        sl = slice(tb * 512, (tb + 1) * 512)
        if tb == 0:
            # fine-grained first block so the first matmuls start early,
            # interleaved with the first stack's weights
            nc.sync.dma_start(wq_sb[:, 0:2, :], wq[0:256, :]
                              .rearrange("(k p) m -> p k m", p=128))
            xts = []
            for k in range(NK):
                xt = xpool.tile([128, 512], F32R, tag="xt")
                nc.sync.dma_start(xt[:], xT[k * 128:(k + 1) * 128, sl])
                xts.append(xt[:])
            nc.sync.dma_start(wq_sb[:, 2:NK, :], wq[256:C, :]
                              .rearrange("(k p) m -> p k m", p=128))
            for w_sb, w_ap in ((wk_sb, wk), (wqr_sb, wqr), (wkr_sb, wkr)):
                nc.sync.dma_start(w_sb[:], w_ap.rearrange("(k p) m -> p k m",
                                                          p=128))
            nc.sync.dma_start(wv_sb[:], wv.rearrange("(k p) m -> p k m",
                                                     p=128))
        else:
            xtb = xbpool.tile([128, NK, 512], F32R, tag="xtb")
            nc.sync.dma_start(xtb[:], xT3[:, :, sl])
            xts = [xtb[:, k, :] for k in range(NK)]
        for w_sb, dst in ((wq_sb, qTf), (wk_sb, kTf),
                          (wqr_sb, qTr), (wkr_sb, kTr)):
            psum = ps_misc.tile([128, 512], F32, tag="m")
            for k in range(NK):
                nc.tensor.matmul(psum[:], w_sb[:, k, :], xts[k],
                                 start=(k == 0), stop=(k == NK - 1))
            nc.vector.tensor_copy(dst[:, sl], psum[:])
        for tt in range(4):
            gt = tb * 4 + tt  # global T-tile
            psv = ps_misc.tile([128, 256], F32, tag="m")
            for k in range(NK):
                nc.tensor.matmul(psv[:], xts[k][:, tt * 128:(tt + 1) * 128],
                                 wv_sb[:, k, :],
                                 start=(k == 0), stop=(k == NK - 1))
            nc.vector.tensor_copy(
                v_sb[:, gt, :, 0:64],
                psv[:].rearrange("p (h d) -> p h d", h=4))

    # w_proj load late: first needed by phase D interleaved into attention
    nc.sync.dma_start(wproj_sb[:], wproj.rearrange("(k p) m -> p k m", p=128))

    # ---- phase C + D interleaved ----
    def attn_block(qT, kT_, Q, W, m_sb, mask_d, heads, yT, is_full, qb):
        i0 = qb * Q
        kt_lo = max(0, i0 - W + 1) // 128
        kt_hi = (i0 + Q - 1) // 128
        kts = list(range(kt_lo, kt_hi + 1))
        py_a = ps_y.tile([128, Q], F32, tag="yA")
        py_b = ps_y.tile([128, Q], F32, tag="yB")
        for idx, kt in enumerate(kts):
            d = i0 - kt * 128
            pss = ps_s.tile([128, 2, 512], F32, tag="s")
            nc.tensor.matmul(pss[:, 0, 0:Q],
                             kT_[0:64, kt * 128:(kt + 1) * 128],
                             qT[0:64, i0:i0 + Q], start=True, stop=True)
            nc.tensor.matmul(pss[:, 1, 0:Q],
                             kT_[64:128, kt * 128:(kt + 1) * 128],
                             qT[64:128, i0:i0 + Q], start=True, stop=True)
            p_sb = ppool.tile([128, 2 * Q], BF16, tag="p")
            nc.scalar.activation(
                p_sb[:].rearrange("p (r q) -> p r q", r=2),
                pss[:, :, 0:Q], mybir.ActivationFunctionType.Exp)
            midx = _full_mask_idx(d) if is_full else mask_d.index(d)
            if midx is not None:
                mm = m_sb[:, midx, :].rearrange(
                    "p (a q) -> p a q", a=1).broadcast_to([128, 2, Q])
                nc.vector.tensor_mul(
                    p_sb[:].rearrange("p (r q) -> p r q", r=2),
                    p_sb[:].rearrange("p (r q) -> p r q", r=2), mm)
            nc.tensor.matmul(py_a[:], v_sb[:, kt, heads[0], :], p_sb[:, 0:Q],
                             start=(idx == 0), stop=(idx == len(kts) - 1))
            nc.tensor.matmul(py_b[:], v_sb[:, kt, heads[1], :], p_sb[:, Q:2 * Q],
                             start=(idx == 0), stop=(idx == len(kts) - 1))
        # normalize: yT rows = py[0:64] * reciprocal(denominator rows)
        for py, rows in ((py_a, slice(0, 64)), (py_b, slice(64, 128))):
            r_sb = rpool.tile([64, Q], F32, tag="r")
            nc.vector.reciprocal(r_sb[:], py[64:128, :])
            nc.vector.tensor_mul(yT[rows, i0:i0 + Q], py[0:64, :], r_sb[:])

    for f in range(T // QF):
        attn_block(qTf, kTf, QF, WF, mf_sb, MASKF_D, (0, 1), yTf, True, f)
        for rqb in (2 * f, 2 * f + 1):
            attn_block(qTr, kTr, QR, WR, mr_sb, MASKR_D, (2, 3), yTr, False, rqb)
        # c_proj for the 4 T-tiles this region covers
        for tt in range(4 * f, 4 * f + 4):
            tsl = slice(tt * 128, (tt + 1) * 128)
            o_sb = opool.tile([128, 1024], F32, tag="osb")
            for nb in range(2):
                nsl = slice(nb * 512, (nb + 1) * 512)
                pso = ps_misc.tile([128, 512], F32, tag="m")
                nc.tensor.matmul(pso[:], yTf[:, tsl], wproj_sb[:, 0, nsl],
                                 start=True, stop=False)
                nc.tensor.matmul(pso[:], yTr[:, tsl], wproj_sb[:, 1, nsl],
                                 start=False, stop=True)
                if nb == 0:
                    nc.scalar.copy(o_sb[:, nsl], pso[:])
                else:
                    nc.vector.tensor_copy(o_sb[:, nsl], pso[:])
            nc.sync.dma_start(out[tsl, :], o_sb[:])


def _build_nc(reps=1):
    nc = bacc.Bacc(trn_type="TRN2", target_bir_lowering=False, debug=False,
                   num_devices=1)

    xT = nc.dram_tensor("xT", [C, T], F32R, kind="ExternalInput").ap()
    wq = nc.dram_tensor("wq", [C, 128], F32R, kind="ExternalInput").ap()
    wk = nc.dram_tensor("wk", [C, 128], F32R, kind="ExternalInput").ap()
    wqr = nc.dram_tensor("wqr", [C, 128], F32R, kind="ExternalInput").ap()
    wkr = nc.dram_tensor("wkr", [C, 128], F32R, kind="ExternalInput").ap()
    wv = nc.dram_tensor("wv", [C, 256], F32R, kind="ExternalInput").ap()
    wproj = nc.dram_tensor("wproj", [256, C], F32R, kind="ExternalInput").ap()
    out = nc.dram_tensor("o", [T, C], F32, kind="ExternalOutput").ap()
    aps = (xT, wq, wk, wqr, wkr, wv, wproj, out)

    with TileContext(nc) as tc:
        with (
            tc.tile_pool(name="wpool", bufs=1) as wpool,
            tc.tile_pool(name="xpool", bufs=10) as xpool,
            tc.tile_pool(name="xbpool", bufs=2) as xbpool,
            tc.tile_pool(name="qk", bufs=1) as qkpool,
            tc.tile_pool(name="ppool", bufs=4) as ppool,
            tc.tile_pool(name="opool", bufs=3) as opool,
            tc.tile_pool(name="rpool", bufs=4) as rpool,
            tc.tile_pool(name="ps_misc", bufs=2, space="PSUM") as ps_misc,
            tc.tile_pool(name="ps_s", bufs=2, space="PSUM") as ps_s,
            tc.tile_pool(name="ps_y", bufs=1, space="PSUM") as ps_y,
        ):
            pools = (wpool, xpool, xbpool, qkpool, ppool, opool, rpool,
                     ps_misc, ps_s, ps_y)
            for _ in range(reps):
                _emit_body(nc, pools, aps)

    nc.compile()
    return nc


_NC_CACHE = {}


def _get_nc(reps=1):
    if reps not in _NC_CACHE:
        _NC_CACHE[reps] = _build_nc(reps)
    return _NC_CACHE[reps]


def make_in_maps(x, w_qkv_full, w_qk_red, w_v_red, w_proj):
    x = np.asarray(x, np.float32)
    w_qkv_full = np.asarray(w_qkv_full, np.float32)
    w_qk_red = np.asarray(w_qk_red, np.float32)
    w_v_red = np.asarray(w_v_red, np.float32)
    w_proj = np.asarray(w_proj, np.float32)
    sf = np.float32(1.0 / np.sqrt(HDIM))
    sr = np.float32(1.0 / np.sqrt(RDIM))
    in_maps = []
    for c in range(N_CORES):
        b, g = divmod(c, 4)
        hA, hB = 2 * g, 2 * g + 1
        wq = np.concatenate([w_qkv_full[:, 64 * hA:64 * hA + 64],
                             w_qkv_full[:, 64 * hB:64 * hB + 64]], 1) * sf
        wk = np.concatenate([w_qkv_full[:, 512 + 64 * hA:512 + 64 * hA + 64],
                             w_qkv_full[:, 512 + 64 * hB:512 + 64 * hB + 64]], 1)
        wqr = np.zeros((C, 128), np.float32)
        wqr[:, 0:32] = w_qk_red[:, 32 * hA:32 * hA + 32] * sr
        wqr[:, 64:96] = w_qk_red[:, 32 * hB:32 * hB + 32] * sr
        wkr = np.zeros((C, 128), np.float32)
        wkr[:, 0:32] = w_qk_red[:, 256 + 32 * hA:256 + 32 * hA + 32]
        wkr[:, 64:96] = w_qk_red[:, 256 + 32 * hB:256 + 32 * hB + 32]
        wv = np.concatenate([w_qkv_full[:, 1024 + 64 * hA:1024 + 64 * hA + 64],
                             w_qkv_full[:, 1024 + 64 * hB:1024 + 64 * hB + 64],
                             w_v_red[:, 64 * hA:64 * hA + 64],
                             w_v_red[:, 64 * hB:64 * hB + 64]], 1)
        wp = np.concatenate([w_proj[64 * hA:64 * hA + 64, :],
                             w_proj[64 * hB:64 * hB + 64, :],
                             w_proj[512 + 64 * hA:512 + 64 * hA + 64, :],
                             w_proj[512 + 64 * hB:512 + 64 * hB + 64, :]], 0)
        in_maps.append({
            "xT": np.ascontiguousarray(x[b].T),
            "wq": np.ascontiguousarray(wq), "wk": np.ascontiguousarray(wk),
            "wqr": wqr, "wkr": wkr, "wv": np.ascontiguousarray(wv),
            "wproj": np.ascontiguousarray(wp),
        })
    return in_maps


def kernel(x, w_qkv_full, w_qk_red, w_v_red, w_proj):
    nc = _get_nc()
    in_maps = make_in_maps(x, w_qkv_full, w_qk_red, w_v_red, w_proj)
    r = bass_utils.run_bass_kernel_spmd(nc, in_maps,
                                        core_ids=list(range(N_CORES)),
                                        trace=False)
    outs = [r.results[c]["o"] for c in range(N_CORES)]
    y = np.zeros((B, T, C), np.float32)
    for b in range(B):
        y[b] = outs[4 * b] + outs[4 * b + 1] + outs[4 * b + 2] + outs[4 * b + 3]
    return y



# revision 10
# speedup vs baseline: 1.3554x; 1.3554x over previous
"""Trainium2 Bass kernel for sparse CausalSelfAttention (8 full heads W=1024,
8 reduced-qk heads W=256), SPMD over 8 NeuronCores.

Sharding: core c -> batch c//4, head-group g=c%4 (full heads 2g,2g+1 and
reduced heads 2g,2g+1). Q=128 attention tiling for both head groups (window
1024 = 8 tiles aligns so interior score tiles are mask-free; the two edge
tiles use complementary 128x128 triangle masks). Projections and c_proj are
woven between attention score groups so PE never waits on the exp chain.
All operands bf16 (psum accumulation f32). Host sums the 4 partial c_proj
outputs per batch element.
"""

import numpy as np

import concourse.bacc as bacc
import concourse.mybir as mybir
from concourse import bass_utils
from concourse.tile import TileContext

# problem constants (hardcoded; kernel.py must be self-contained)
B, T, C = 2, 2048, 1024
HDIM = 64          # full head dim (and v dim of reduced heads)
RDIM = 32          # reduced qk dim
NTT = T // 128     # 16 query tiles of 128
NREG = T // 512    # 4 regions of 4 tiles
N_CORES = 8
NK = C // 128      # k-tiles over C contraction

F32 = mybir.dt.float32
BF16 = mybir.dt.bfloat16

FULL_SPAN = 8      # window 1024 / 128
RED_SPAN = 2       # window 256 / 128


def _make_masks(nc, m_sb):
    """m_sb[:, 0] = lower-strict triangle (f < p); m_sb[:, 1] = upper (f >= p)."""
    nc.gpsimd.memset(m_sb[:], 1.0)
    # m_lo: keep where (-1 - f + p) >= 0
    nc.gpsimd.affine_select(out=m_sb[:, 0, :], in_=m_sb[:, 0, :],
                            compare_op=mybir.AluOpType.is_ge, fill=0.0,
                            base=-1, pattern=[[-1, 128]], channel_multiplier=1)
    # m_hi: keep where (f - p) >= 0
    nc.gpsimd.affine_select(out=m_sb[:, 1, :], in_=m_sb[:, 1, :],
                            compare_op=mybir.AluOpType.is_ge, fill=0.0,
                            base=0, pattern=[[1, 128]], channel_multiplier=-1)


class Emitter:
    def __init__(self, nc, pools, aps):
        self.nc = nc
        (self.wpool, self.xpool, self.xbpool, self.qkpool, self.ppool,
         self.opool, self.rpool, self.ps_misc, self.ps_s, self.ps_y) = pools
        (self.xT, self.wq, self.wk, self.wqkr, self.wv, self.wproj,
         self.out) = aps

    def setup_tiles(self):
        nc = self.nc
        w = self.wpool
        self.wq_sb = w.tile([128, NK, 128], BF16, tag="wq")
        self.wk_sb = w.tile([128, NK, 128], BF16, tag="wk")
        self.wqkr_sb = w.tile([128, NK, 128], BF16, tag="wqkr")
        self.wv_sb = w.tile([128, NK, 256], BF16, tag="wv")
        self.wproj_sb = w.tile([128, 2, C], BF16, tag="wproj")
        self.m_sb = w.tile([128, 2, 128], BF16, tag="m")
        _make_masks(nc, self.m_sb)

        qk = self.qkpool
        # q tiles are zero-padded per head so every score matmul contracts the
        # full partition range at tile_position (0,0): bf16 matmuls crash when
        # consecutive instructions change partition offset (HW-probed)
        self.qTfA = qk.tile([128, T], BF16, tag="qTfA")
        self.qTfB = qk.tile([128, T], BF16, tag="qTfB")
        nc.gpsimd.memset(self.qTfA[64:128, :], 0.0)
        nc.gpsimd.memset(self.qTfB[0:64, :], 0.0)
        self.kTf = qk.tile([128, T], BF16, tag="kTf")
        self.qTrA = qk.tile([64, T], BF16, tag="qTrA")
        self.qTrB = qk.tile([64, T], BF16, tag="qTrB")
        nc.gpsimd.memset(self.qTrA[32:64, :], 0.0)
        nc.gpsimd.memset(self.qTrB[0:32, :], 0.0)
        self.kTr = qk.tile([64, T], BF16, tag="kTr")
        self.v_sb = qk.tile([128, NTT, 4, 128], BF16, tag="v")
        nc.gpsimd.memset(self.v_sb[:, :, :, 64:128], 1.0)
        self.yTf = qk.tile([128, T], BF16, tag="yTf")
        self.yTr = qk.tile([128, T], BF16, tag="yTr")

    # ---- projections ----------------------------------------------------
    def prologue_dma_and_proj0(self):
        """Block-0 x DMA interleaved with weight DMAs + projections for
        region 0, emitted directly (nothing to weave against yet)."""
        nc = self.nc
        nc.sync.dma_start(self.wq_sb[:, 0:2, :],
                          self.wq[0:256, :].rearrange("(k p) m -> p k m", p=128))
        xts = []
        for k in range(NK):
            xt = self.xpool.tile([128, 512], BF16, tag="xt")
            nc.sync.dma_start(xt[:], self.xT[k * 128:(k + 1) * 128, 0:512])
            xts.append(xt[:])
        nc.sync.dma_start(self.wq_sb[:, 2:NK, :],
                          self.wq[256:C, :].rearrange("(k p) m -> p k m", p=128))
        for w_sb, w_ap in ((self.wk_sb, self.wk), (self.wqkr_sb, self.wqkr),
                           (self.wv_sb, self.wv)):
            nc.sync.dma_start(w_sb[:], w_ap.rearrange("(k p) m -> p k m", p=128))
        self.xbs = [None]
        for f in range(1, NREG):
            xb = self.xbpool.tile([128, NK, 512], BF16, tag="xb")
            self.xbs.append(xb)

        def xb_dma(f):
            nc.sync.dma_start(
                self.xbs[f][:], self.xT.rearrange("(k p) t -> p k t", p=128)
                [:, :, f * 512:(f + 1) * 512])
        xb_dma(1)
        nc.sync.dma_start(self.wproj_sb[:],
                          self.wproj.rearrange("(k p) m -> p k m", p=128))
        xb_dma(2)
        xb_dma(3)
        for u in self.proj_units(0, xts):
            u()

    def proj_units(self, f, xts=None):
        """Filler units projecting x block f into qTf/kTf/qTr/kTr/v_sb."""
        nc = self.nc
        if xts is None:
            xb = self.xbs[f]
            xts = [xb[:, k, :] for k in range(NK)]
        sl = slice(f * 512, (f + 1) * 512)
        units = []

        def slab(w_sb, do_copy):
            psq = [None]

            def mk(k0):
                def u():
                    if k0 == 0:
                        psq[0] = self.ps_misc.tile([128, 512], F32, tag="m", name="psq")
                    for k in (k0, k0 + 1):
                        nc.tensor.matmul(psq[0][:], w_sb[:, k, :], xts[k],
                                         start=(k == 0), stop=(k == NK - 1))
                    if k0 == NK - 2:
                        do_copy(psq[0])
                return u
            return [mk(k0) for k0 in range(0, NK, 2)]

        def q_copy(ps):
            nc.scalar.copy(self.qTfA[0:64, sl], ps[0:64, :])
            nc.scalar.copy(self.qTfB[64:128, sl], ps[64:128, :])
        units += slab(self.wq_sb, q_copy)
        units += slab(self.wk_sb,
                      lambda ps: nc.scalar.copy(self.kTf[:, sl], ps[:]))

        def qkr_copy(ps):
            nc.vector.tensor_copy(self.qTrA[0:32, sl], ps[0:32, :])
            nc.vector.tensor_copy(self.qTrB[32:64, sl], ps[32:64, :])
            nc.vector.tensor_copy(self.kTr[:, sl], ps[64:128, :])
        units += slab(self.wqkr_sb, qkr_copy)

        for tt in range(4 * f, 4 * f + 4):
            psv = [None]

            def mkv(tt, k0):
                def u():
                    if k0 == 0:
                        psv[0] = self.ps_misc.tile([128, 256], F32, tag="m", name="psv")
                    for k in range(k0, k0 + 4):
                        nc.tensor.matmul(
                            psv[0][:],
                            xts[k][:, (tt % 4) * 128:(tt % 4) * 128 + 128],
                            self.wv_sb[:, k, :],
                            start=(k == 0), stop=(k == NK - 1))
                    if k0 == 4:
                        eng = nc.vector.tensor_copy if tt % 2 else nc.scalar.copy
                        eng(self.v_sb[:, tt, :, 0:64],
                            psv[0][:].rearrange("p (h d) -> p h d", h=4))
                return u
            units += [mkv(tt, 0), mkv(tt, 4)]
        return units

    # ---- attention ------------------------------------------------------
    def attn_qb_units(self, qb, is_full):
        """Units for one 128-query block: score groups (shared exp) + PV
        accumulation + normalization."""
        nc = self.nc
        span = FULL_SPAN if is_full else RED_SPAN
        if is_full:
            qTs, kT, krows = (self.qTfA, self.qTfB), self.kTf, slice(0, 128)
        else:
            qTs, kT, krows = (self.qTrA, self.qTrB), self.kTr, slice(0, 64)
        yT = self.yTf if is_full else self.yTr
        vh0 = 0 if is_full else 2
        ks = list(range(max(0, qb - span), qb + 1))
        groups = []
        i = len(ks)
        while i > 0:
            groups.insert(0, ks[max(0, i - 4):i])
            i -= 4
        qsl = slice(qb * 128, (qb + 1) * 128)
        py = [None]
        shared = {}
        units = []

        def mk_scores(g, first):
            def u():
                if first:
                    # heads in separate psum banks (h stride = 2KB): each
                    # head's PV chain must be sole writer of its bank
                    py[0] = self.ps_y.tile([128, 2, 512], F32, tag="y",
                                           name="py")
                glen = len(g)
                pss = self.ps_s.tile([128, glen, 2, 128], F32, tag="s",
                                     name="pss")
                shared["pss"] = pss
                for i, kt in enumerate(g):
                    ksl = slice(kt * 128, (kt + 1) * 128)
                    for h in range(2):
                        # full contraction vs zero-padded q: keeps every
                        # matmul at tile_position (0,0); each strip written
                        # exactly once with its own start+stop
                        nc.tensor.matmul(
                            pss[:, i, h, :], kT[krows, ksl], qTs[h][krows, qsl],
                            start=True, stop=True)
                p_sb = self.ppool.tile([128, glen, 2, 128], BF16, tag="p",
                                       name="psb")
                shared["p_sb"] = p_sb
                nc.scalar.activation(p_sb[:], pss[:],
                                     mybir.ActivationFunctionType.Exp)
                for i, kt in enumerate(g):
                    midx = 0 if kt == qb - span else (1 if kt == qb else None)
                    if midx is not None:
                        mm = self.m_sb[:, midx, :].rearrange(
                            "p (a q) -> p a q", a=1).broadcast_to([128, 2, 128])
                        nc.vector.tensor_mul(p_sb[:, i, :, :],
                                             p_sb[:, i, :, :], mm)
            return u

        def mk_pv(g, first, last):
            def u():
                p_sb = shared["p_sb"]
                for i, kt in enumerate(g):
                    for h in range(2):
                        nc.tensor.matmul(py[0][:, h, 0:128],
                                         self.v_sb[:, kt, vh0 + h, :],
                                         p_sb[:, i, h, :],
                                         start=(first and i == 0),
                                         stop=(last and i == len(g) - 1))
                if last:
                    r_sb = self.rpool.tile([64, 2, 128], F32, tag="r")
                    nc.vector.reciprocal(r_sb[:], py[0][64:128, :, 0:128])
                    for h in range(2):
                        nc.vector.tensor_mul(yT[h * 64:(h + 1) * 64, qsl],
                                             py[0][0:64, h, 0:128],
                                             r_sb[:, h, :])
            return u

        for gi, g in enumerate(groups):
            units.append(mk_scores(g, gi == 0))
            units.append(mk_pv(g, gi == 0, gi == len(groups) - 1))
        return units

    # ---- c_proj ---------------------------------------------------------
    def cproj_units(self, f):
        nc = self.nc
        units = []
        for tt in range(4 * f, 4 * f + 4):
            tsl = slice(tt * 128, (tt + 1) * 128)
            o_sb = [None]

            def mk(tt, tsl, nb):
                def u():
                    if nb == 0:
                        o_sb[0] = self.opool.tile([128, C], BF16, tag="o", name="osb")
                    nsl = slice(nb * 512, (nb + 1) * 512)
                    pso = self.ps_misc.tile([128, 512], F32, tag="m")
                    nc.tensor.matmul(pso[:], self.yTf[:, tsl],
                                     self.wproj_sb[:, 0, nsl],
                                     start=True, stop=False)
                    nc.tensor.matmul(pso[:], self.yTr[:, tsl],
                                     self.wproj_sb[:, 1, nsl],
                                     start=False, stop=True)
                    if nb == 0:
                        nc.scalar.copy(o_sb[0][:, nsl], pso[:])
                    else:
                        nc.vector.tensor_copy(o_sb[0][:, nsl], pso[:])
                        nc.sync.dma_start(self.out[tsl, :], o_sb[0][:])
                return u
            units += [mk(tt, tsl, 0), mk(tt, tsl, 1)]
        return units

    # ---- weave ----------------------------------------------------------
    def region(self, f, fillers, inline_cproj=False):
        units = []
        cp = self.cproj_units(f) if inline_cproj else [None] * 8
        for i, qb in enumerate(range(4 * f, 4 * f + 4)):
            units += self.attn_qb_units(qb, True)
            units += self.attn_qb_units(qb, False)
            if inline_cproj:
                units += cp[2 * i:2 * i + 2]
        if inline_cproj:
            units += cp[2:4] and []  # cp consumed above per tt
        fi = 0
        for i, u in enumerate(units):
            u()
            want = (i + 1) * len(fillers) // len(units)
            while fi < want:
                fillers[fi]()
                fi += 1

    def emit(self):
        self.setup_tiles()
        self.prologue_dma_and_proj0()
        self.region(0, self.proj_units(1))
        self.region(1, self.proj_units(2) + self.cproj_units(0))
        self.region(2, self.proj_units(3) + self.cproj_units(1))
        self.region(3, self.cproj_units(2), inline_cproj=True)


def _build_nc(reps=1):
    nc = bacc.Bacc(trn_type="TRN2", target_bir_lowering=False, debug=False,
                   num_devices=1)

    xT = nc.dram_tensor("xT", [C, T], BF16, kind="ExternalInput").ap()
    wq = nc.dram_tensor("wq", [C, 128], BF16, kind="ExternalInput").ap()
    wk = nc.dram_tensor("wk", [C, 128], BF16, kind="ExternalInput").ap()
    wqkr = nc.dram_tensor("wqkr", [C, 128], BF16, kind="ExternalInput").ap()
    wv = nc.dram_tensor("wv", [C, 256], BF16, kind="ExternalInput").ap()
    wproj = nc.dram_tensor("wproj", [256, C], BF16, kind="ExternalInput").ap()
    out = nc.dram_tensor("o", [T, C], BF16, kind="ExternalOutput").ap()
    aps = (xT, wq, wk, wqkr, wv, wproj, out)

    with TileContext(nc) as tc:
        with (
            tc.tile_pool(name="wpool", bufs=1) as wpool,
            tc.tile_pool(name="xpool", bufs=10) as xpool,
            tc.tile_pool(name="xbpool", bufs=3) as xbpool,
            tc.tile_pool(name="qk", bufs=1) as qkpool,
            tc.tile_pool(name="ppool", bufs=4) as ppool,
            tc.tile_pool(name="opool", bufs=3) as opool,
            tc.tile_pool(name="rpool", bufs=4) as rpool,
            tc.tile_pool(name="ps_misc", bufs=2, space="PSUM") as ps_misc,
            tc.tile_pool(name="ps_s", bufs=2, space="PSUM") as ps_s,
            tc.tile_pool(name="ps_y", bufs=1, space="PSUM") as ps_y,
        ):
            pools = (wpool, xpool, xbpool, qkpool, ppool, opool, rpool,
                     ps_misc, ps_s, ps_y)
            for _ in range(reps):
                Emitter(nc, pools, aps).emit()

    nc.compile()
    return nc


_NC_CACHE = {}


def _get_nc(reps=1):
    if reps not in _NC_CACHE:
        _NC_CACHE[reps] = _build_nc(reps)
    return _NC_CACHE[reps]


def make_in_maps(x, w_qkv_full, w_qk_red, w_v_red, w_proj):
    import ml_dtypes
    bf = ml_dtypes.bfloat16
    x = np.asarray(x, np.float32)
    w_qkv_full = np.asarray(w_qkv_full, np.float32)
    w_qk_red = np.asarray(w_qk_red, np.float32)
    w_v_red = np.asarray(w_v_red, np.float32)
    w_proj = np.asarray(w_proj, np.float32)
    sf = np.float32(1.0 / np.sqrt(HDIM))
    sr = np.float32(1.0 / np.sqrt(RDIM))
    in_maps = []
    for c in range(N_CORES):
        b, g = divmod(c, 4)
        hA, hB = 2 * g, 2 * g + 1
        wq = np.concatenate([w_qkv_full[:, 64 * hA:64 * hA + 64],
                             w_qkv_full[:, 64 * hB:64 * hB + 64]], 1) * sf
        wk = np.concatenate([w_qkv_full[:, 512 + 64 * hA:512 + 64 * hA + 64],
                             w_qkv_full[:, 512 + 64 * hB:512 + 64 * hB + 64]], 1)
        wqkr = np.concatenate(
            [w_qk_red[:, 32 * hA:32 * hA + 32] * sr,
             w_qk_red[:, 32 * hB:32 * hB + 32] * sr,
             w_qk_red[:, 256 + 32 * hA:256 + 32 * hA + 32],
             w_qk_red[:, 256 + 32 * hB:256 + 32 * hB + 32]], 1)
        wv = np.concatenate([w_qkv_full[:, 1024 + 64 * hA:1024 + 64 * hA + 64],
                             w_qkv_full[:, 1024 + 64 * hB:1024 + 64 * hB + 64],
                             w_v_red[:, 64 * hA:64 * hA + 64],
                             w_v_red[:, 64 * hB:64 * hB + 64]], 1)
        wp = np.concatenate([w_proj[64 * hA:64 * hA + 64, :],
                             w_proj[64 * hB:64 * hB + 64, :],
                             w_proj[512 + 64 * hA:512 + 64 * hA + 64, :],
                             w_proj[512 + 64 * hB:512 + 64 * hB + 64, :]], 0)
        in_maps.append({
            "xT": np.ascontiguousarray(x[b].T).astype(bf),
            "wq": np.ascontiguousarray(wq).astype(bf),
            "wk": np.ascontiguousarray(wk).astype(bf),
            "wqkr": np.ascontiguousarray(wqkr).astype(bf),
            "wv": np.ascontiguousarray(wv).astype(bf),
            "wproj": np.ascontiguousarray(wp).astype(bf),
        })
    return in_maps


def kernel(x, w_qkv_full, w_qk_red, w_v_red, w_proj):
    nc = _get_nc()
    in_maps = make_in_maps(x, w_qkv_full, w_qk_red, w_v_red, w_proj)
    r = bass_utils.run_bass_kernel_spmd(nc, in_maps,
                                        core_ids=list(range(N_CORES)),
                                        trace=False)
    outs = [np.asarray(r.results[c]["o"], dtype=np.float32)
            for c in range(N_CORES)]
    y = np.zeros((B, T, C), np.float32)
    for b in range(B):
        y[b] = outs[4 * b] + outs[4 * b + 1] + outs[4 * b + 2] + outs[4 * b + 3]
    return y
